# revision 28
# baseline (speedup 1.0000x reference)
"""GraphShiftOperator on 8 Trainium2 NeuronCores (raw Bass, explicit sync).

reference:
    out_deg = A.sum(1); in_deg = A.sum(0)
    forward = A.T * (1/(out_deg+eps))[None, :]   # = (diag(1/out_deg) @ A).T
    reverse = A  * (1/(in_deg+eps))[None, :]

v2 design (vs the fp32 two-pass v1 at 551 us):
  * All device I/O in bf16 (correctness gate is rel_err < 2e-2; bf16
    costs ~0.5%).  Halves every DMA transfer.
  * The whole 16 MB bf16 row-shard stays resident in SBUF, so A is
    read from HBM ONCE (v1 re-read it for the reverse pass).
    Per-core HBM traffic: 16 MB in + 32 MB out = 48 MB (vs 128 MB).
  * Column sums on PE with a ones[128,1] STATIONARY and 512-wide
    moving slices of A: 128 matmuls total (vs 512 chunk-stationary
    matmuls whose weight reloads made PE the 219 us critical path).
    Compact [1,512] PSUM rows come out in natural column order, so
    v1's PE transpose + its PSUM juggling are gone entirely.
    PSUM only holds 8 such banks -> columns 0..4095 accumulate while
    tiles stream in (phase A), columns 4096..8191 in a short PE-only
    re-pass over SBUF after the last load (phase B).
  * AllReduce (32 KB fp32) launches right after phase B and overlaps
    the tail of the forward stores.
  * d_in reciprocal is computed on the COMPACT [128,64] form before
    broadcast (v1 spent 51 us reciprocating the broadcast tile).

Per core:
  fwd_scaled = A_s * d_out_inv[:, None]   (host: forward = vstack.T)
  rev        = A_s * d_in_inv[None, :]    (rows of `reverse`)

Engine split:
  SP(sync)  A-tile loads, colsum bounce, d_in chain DMAs
  ACT       fwd multiplies, fwd + rev output stores
  DVE       row sums, reciprocals, PSUM->SBUF colsum copies, rev muls
  PE        column sums (ones-stationary matmuls)
  GPSIMD    AllReduce
"""

import sys

sys.path.insert(0, "/opt/trn_rl_repo")

from contextlib import ExitStack

import numpy as np
import ml_dtypes

import concourse.bass as bass
from concourse import mybir
from concourse.bass_utils import run_bass_kernel_spmd

N = 8192
N_CORES = 8
ROWS = N // N_CORES          # 1024 rows per core
P = 128                      # partitions
NT = ROWS // P               # 8 row-tiles per core
CH = 512                     # moving-dim chunk (PE max)
NCH = N // CH                # 16 column chunks
HALF = NCH // 2              # 8 chunks per PSUM phase
DT = mybir.dt.bfloat16
F32 = mybir.dt.float32

_cache = {}


def _build():
    nc = bass.Bass(num_devices=N_CORES)

    a_in = nc.dram_tensor("a_shard", [ROWS, N], DT, kind="ExternalInput")
    fwd_out = nc.dram_tensor("fwd_scaled", [ROWS, N], DT, kind="ExternalOutput")
    rev_out = nc.dram_tensor("rev", [ROWS, N], DT, kind="ExternalOutput")
    cc_in = nc.dram_tensor("cc_in", [N], F32)
    cc_ag = nc.dram_tensor("cc_ag", [N_CORES * N], F32)
    din_c = nc.dram_tensor("din_c", [N], DT)

    ctx = ExitStack()
    with ctx:
        sem = lambda name: ctx.enter_context(nc.semaphore(name))
        li = sem("li")      # A-tile loads            (+16 each)
        on = sem("on")      # ones memset + psum zero done
        dv1 = sem("dv1")    # doi ready               (+1 per tile)
        am = sem("am")      # ACT fwd multiply done   (+1 per tile)
        fo = sem("fo")      # fwd store done          (+16 each)
        pA = sem("pA")      # colsum chunk finished   (+1 at stop)
        cpA = sem("cpA")    # colsum psum copy done
        cci = sem("cci")    # colsum bounce DMA       (+16)
        cc = sem("cc")      # collective done
        cmpi = sem("cmpi")  # cc_out compact load     (+16)
        rcp = sem("rcp")    # compact reciprocal done
        dco = sem("dco")    # din_c store             (+16)
        dinb = sem("dinb")  # d_in broadcast          (+16)
        dv2 = sem("dv2")    # rev multiply done       (+1 per tile)
        ro = sem("ro")      # rev store done          (+16 each)

        sb = lambda name, shape, dt=DT: ctx.enter_context(
            nc.sbuf_tensor(name, shape, dt)
        )
        H = N // 2
        a_sb = [sb(f"a{t}", [P, N]) for t in range(NT)]   # resident shard
        f_sb = [sb(f"f{i}", [P, N]) for i in range(2)]    # fwd staging
        junk = sb("junk", [P, H])                         # ttr main output
        din = sb("din", [P, N])                           # d_in_inv bcast
        ones = sb("ones", [P, 1])
        rs = sb("rs", [P, 1], F32)
        doi = [sb(f"doi{t}", [P, 1], F32) for t in range(NT)]
        cs = sb("cs", [65, HALF * CH], F32)               # compact colsums
        agg = sb("agg", [P, N_CORES * (N // P)], F32)     # gathered partials
        cmp_f = sb("cmp_f", [P, N // P], F32)             # compact in_deg
        cmp_b = sb("cmp_b", [P, N // P], DT)

        # chunk c lives at (partition (c//8)*64, bank c%8): all 16 colsum
        # chunks accumulate simultaneously in the 8 PSUM banks (PE output
        # base partition must be 0/32/64)
        ps = ctx.enter_context(nc.psum_tensor("ps", [P, HALF * CH], F32))

        with nc.Block() as block:

            @block.sync
            def _(sync):
                for t in range(NT):
                    sync.dma_start(
                        out=a_sb[t][:], in_=a_in[t * P : (t + 1) * P, :]
                    ).then_inc(li, 16)
                # compact colsums -> collective input (rows 0 and 64)
                sync.wait_ge(cpA, 1)
                sync.dma_start(
                    out=cc_in[0 : HALF * CH], in_=cs[0:1, :]
                ).then_inc(cci, 16)
                sync.dma_start(
                    out=cc_in[HALF * CH : N], in_=cs[64:65, :]
                ).then_inc(cci, 16)
                # d_in chain: load gathered partials, store bf16, broadcast
                sync.wait_ge(cc, 1)
                sync.dma_start(
                    out=agg[:],
                    in_=bass.AP(cc_ag, 0, [[N // P, P], [N, N_CORES], [1, N // P]]),
                ).then_inc(cmpi, 16)
                sync.wait_ge(rcp, 1)
                sync.dma_start(
                    out=bass.AP(din_c, 0, [[N // P, P], [1, N // P]]),
                    in_=cmp_b[:],
                ).then_inc(dco, 16)
                sync.wait_ge(dco, 16)
                sync.dma_start(
                    out=din[:],
                    in_=bass.AP(din_c, 0, [[0, P], [1, N]]),
                ).then_inc(dinb, 16)

            @block.scalar
            def _(scalar):
                # fwd multiply, double-buffered.  Stores are GATED on all
                # loads having landed: loads then get the full HBM
                # bandwidth (measured line-rate when running alone), which
                # pulls the whole colsum->AllReduce->d_in chain ~25us
                # earlier.  The store bandwidth is repaid during the
                # AllReduce + d_in window, which is latency-bound anyway.
                for t in range(NT):
                    scalar.wait_ge(li, 16 * (t + 1))
                    scalar.wait_ge(dv1, t + 1)
                    if t >= 2:
                        scalar.wait_ge(fo, 16 * (t - 1))  # buf t-2 stored
                    scalar.mul(f_sb[t % 2][:], a_sb[t][:], doi[t][:])
                    scalar.drain().then_inc(am, 1)
                    if t == 0:
                        scalar.wait_ge(li, 16 * NT)       # store gate
                    scalar.dma_start(
                        out=fwd_out[t * P : (t + 1) * P, :], in_=f_sb[t % 2][:]
                    ).then_inc(fo, 16)
                for t in range(NT):
                    scalar.wait_ge(dv2, t + 1)
                    scalar.dma_start(
                        out=rev_out[t * P : (t + 1) * P, :], in_=a_sb[t][:]
                    ).then_inc(ro, 16)

            @block.vector
            def _(vector):
                # NOTE: raw bass needs explicit drains for DVE results to
                # become visible (Tile inserts these automatically).
                vector.memset(ones[:], 1.0)
                vector.memset(ps[:], 0.0)
                vector.drain().then_inc(on, 1)
                for t in range(NT):
                    vector.wait_ge(li, 16 * (t + 1))
                    # row sums: two bf16 TT folds (2 elem/cyc) + short
                    # reduce -- 5.1k cycles instead of 8.2k for a flat
                    # reduce.  Folds run in bf16 (error ~2 ulp per 4-sum,
                    # washed out by the fp32 final accumulate).
                    vector.tensor_add(junk[:], a_sb[t][:, 0:H], a_sb[t][:, H:N])
                    vector.drain()
                    vector.tensor_add(
                        junk[:, 0 : H // 2], junk[:, 0 : H // 2], junk[:, H // 2 : H]
                    )
                    vector.drain()
                    vector.reduce_sum(
                        out=rs[:],
                        in_=junk[:, 0 : H // 2],
                        axis=mybir.AxisListType.X,
                    )
                    vector.drain()
                    vector.reciprocal(doi[t][:], rs[:])
                    vector.drain().then_inc(dv1, 1)
                # compact colsum copy: partitions 0..64 in one parallel op
                # (only rows 0 and 64 carry data; lanes run in parallel so
                # copying the span costs the same 4k cycles)
                vector.wait_ge(pA, NCH)
                vector.tensor_copy(cs[:], ps[0:65, :])
                vector.drain().then_inc(cpA, 1)
                # sum the 8 gathered partial rows, then reciprocal -> bf16
                vector.wait_ge(cmpi, 16)
                K = N // P
                vector.tensor_add(cmp_f[:], agg[:, 0:K], agg[:, K : 2 * K])
                vector.drain()
                for r in range(2, N_CORES):
                    vector.tensor_add(
                        cmp_f[:], cmp_f[:], agg[:, r * K : (r + 1) * K]
                    )
                    vector.drain()
                with nc.allow_low_precision("d_in_inv rounds to bf16 anyway"):
                    vector.reciprocal(cmp_b[:], cmp_f[:])
                vector.drain().then_inc(rcp, 1)
                # reverse multiplies, in place on the resident tiles
                vector.wait_ge(dinb, 16)
                for t in range(NT):
                    vector.wait_ge(am, t + 1)  # ACT done reading a_sb[t]
                    vector.tensor_mul(a_sb[t][:], a_sb[t][:], din[:])
                    vector.drain().then_inc(dv2, 1)

            @block.tensor
            def _(tensor):
                tensor.wait_ge(on, 1)
                # all 16 column chunks accumulate as tiles land; chunk c
                # targets (partition c//8, bank c%8).  start=False always:
                # start=True zeroes the WHOLE bank, which would wipe the
                # co-resident chunk on the other partition row -- the
                # accumulator is memset once by DVE instead.
                for t in range(NT):
                    tensor.wait_ge(li, 16 * (t + 1))
                    for c in range(NCH):
                        row, b = (c // HALF) * 64, c % HALF
                        mm = tensor.matmul(
                            ps[row : row + 1, b * CH : (b + 1) * CH],
                            ones[:],
                            a_sb[t][:, c * CH : (c + 1) * CH],
                            start=False,
                            stop=(t == NT - 1),
                            skip_group_check=True,
                        )
                        if t == NT - 1:
                            mm.then_inc(pA, 1)

            @block.gpsimd
            def _(gpsimd):
                gpsimd.wait_ge(cci, 32)
                # AllGather + local sum instead of AllReduce: AR is
                # internally RS+AG, so AG alone is ~half the latency;
                # the 7 adds on the compact [128,64] form are ~free.
                gpsimd.collective_compute(
                    "AllGather",
                    mybir.AluOpType.bypass,
                    replica_groups=[list(range(N_CORES))],
                    ins=[cc_in[:]],
                    outs=[cc_ag[:]],
                ).then_inc(cc, 1)

    return nc


def kernel(adjacency_matrix: np.ndarray, _trace=False, _trace_kwargs=None):
    a = np.asarray(adjacency_matrix)
    assert a.shape == (N, N)
    a_bf = np.ascontiguousarray(a).astype(ml_dtypes.bfloat16)

    if "nc" not in _cache:
        _cache["nc"] = _build()
    nc = _cache["nc"]

    in_maps = [
        {"a_shard": a_bf[s * ROWS : (s + 1) * ROWS, :]} for s in range(N_CORES)
    ]
    kw = {}
    if _trace:
        kw = dict(trace=True, **(_trace_kwargs or {}))
    res = run_bass_kernel_spmd(nc, in_maps, list(range(N_CORES)), **kw)

    scaled = np.concatenate([r["fwd_scaled"] for r in res.results], axis=0)
    reverse = np.concatenate([r["rev"] for r in res.results], axis=0)
    forward = scaled.T.astype(np.float32)
    reverse = reverse.astype(np.float32)
    if _trace:
        return (forward, reverse), res
    return forward, reverse
